# revision 18
# baseline (speedup 1.0000x reference)
"""Trainium2 Bass kernel for nn_ActuatorNet (20-layer tiny MLP, softsign).

Strategy (pure data parallel, 8 cores, batch 1048576 -> 131072 rows/core):
  - Activations kept TRANSPOSED in SBUF: features on partitions, rows on free
    dim.  4 partition strips (32 each) process 4 independent row-blocks
    concurrently on the PE's diagonal 32x32 tiles (tile_position=(32i,32i)).
  - Each "oct" = 8 blocks of 512 rows = [128, 1024] tiles (2 PSUM banks).
  - Per layer: 8 matmuls (bf16, fp32 psum) -> ScalarE computes |z+b| (Abs with
    per-partition bias in the free affine) -> one fused custom-DVE op computes
    softsign: y = (z+b) * recip(1+|z+b|) via the BITWISE_NOT reciprocal seed
    plus a linear minimax refinement, writing bf16 for the next matmul.
  - Layer 1 runs in fp32 straight from the DMA'd x (K=6).
  - Final layer: M=1 matmuls + ScalarE Identity(+bout), DMA out.
"""

import os
import re
import sys

import numpy as np

sys.path.insert(0, "/opt/trn_rl_repo")

N_CORES = 8
B_FULL = 1048576
SHARD = B_FULL // N_CORES  # 131072
NBLK = 512                 # rows per block = one psum bank of fp32
OCT_ROWS = 8 * NBLK        # 4096 rows per oct tile [128, 1024]
N_HID = 19

# minimax fit of 1/t ~ A + B*t over t = d*bitcast(~d) in [-4.5, -4]
A_FIT = float(np.float32(-0.4714035350548651))
B_FIT = float(np.float32(-0.05545919627798768))

SOFTSIGN_OP_NAME = "SOFTSIGN_ANT_ACTNET"

LAST_RESULT = None  # BassKernelResults of the most recent run (for test.py)

_cache = {}


def _register_softsign_op():
    """Fused softsign: out = (Src0+C0) * ~d * (C2 + C1 * d*~d), d = Src1+1.

    Src0 = z (psum fp32), Src1 = |z+b| (from ScalarE), C0 = bias [P,1] AP.
    Exactly 8 ALU stages.
    """
    from concourse import dve_ops
    from concourse.dve_spec import AluOp, Bin, C0, C1, C2, One, Spec, Src0, Src1

    if SOFTSIGN_OP_NAME in dve_ops.CUSTOM_DVE_SPECS:
        return next(o for o in dve_ops.OPS if o.name == SOFTSIGN_OP_NAME)

    _zb = Src0 + C0
    _d = Src1 + One
    _nd = Bin(AluOp.BITWISE_NOT, _d, _d)
    _t = _d * _nd
    _s = C2 + _t * C1
    _p = _zb * _nd
    body = _p * _s

    def _ref(in0, in1, s0, s1, imm2):
        zb = in0.astype(np.float32) + np.asarray(s0, np.float32)
        d = (in1.astype(np.float32) + np.float32(1.0)).astype(np.float32)
        nd = (~d.view(np.int32)).view(np.float32)
        t = (d * nd).astype(np.float32)
        s = np.float32(imm2) + t * np.asarray(s1, np.float32)
        return (zb * nd) * s

    spec = Spec(body=body, reference=_ref)
    op = dve_ops.DveOp(SOFTSIGN_OP_NAME, spec, subdim=False, uops_sha={})
    dve_ops._SUB_OPCODE_FOR_NAME[SOFTSIGN_OP_NAME] = (
        max(dve_ops._SUB_OPCODE_FOR_NAME.values()) + 1
    )
    assert dve_ops._SUB_OPCODE_FOR_NAME[SOFTSIGN_OP_NAME] < 0x20
    dve_ops.OPS.append(op)
    dve_ops.CUSTOM_DVE_SPECS[SOFTSIGN_OP_NAME] = spec
    # self-pin the uops sha (computed from our own lower() output)
    for ver in ("v3", "v4"):
        try:
            op.compile(ver)
        except ValueError as e:
            m = re.search(rf"{ver}: ([0-9a-f]{{16}})", str(e))
            if not m:
                raise
            op.uops_sha[ver] = m.group(1)
            op.compile(ver)
    return op


def _build(shard_rows):
    from concourse import bacc, mybir, tile

    ssop = _register_softsign_op()

    f32 = mybir.dt.float32
    fp16 = mybir.dt.float16
    Act = mybir.ActivationFunctionType

    assert shard_rows % OCT_ROWS == 0
    n_oct = shard_rows // OCT_ROWS

    nc = bacc.Bacc()
    x_e = nc.declare_dram_parameter("xq", [24, shard_rows // 4], f32, isOutput=False)
    w1_e = nc.declare_dram_parameter("w1q", [128, 32], f32, isOutput=False)
    wh_e = nc.declare_dram_parameter("whq", [128, N_HID * 32], fp16, isOutput=False)
    wo_e = nc.declare_dram_parameter("woq", [128, 1], fp16, isOutput=False)
    wob_e = nc.declare_dram_parameter("wob", [128, 4], fp16, isOutput=False)
    bq_e = nc.declare_dram_parameter("bq", [128, 20], f32, isOutput=False)
    bo_e = nc.declare_dram_parameter("boq", [128, 1], f32, isOutput=False)
    out_e = nc.declare_dram_parameter("out", [shard_rows, 1], f32, isOutput=True)

    with tile.TileContext(nc) as tc:
        with (
            tc.tile_pool(name="const", bufs=1) as cpool,
            tc.tile_pool(name="xs", bufs=5) as xpool,
            tc.tile_pool(name="h", bufs=9) as hpool,
            tc.tile_pool(name="a", bufs=5) as apool,
            tc.tile_pool(name="ot", bufs=4) as opool,
            tc.tile_pool(name="ps", bufs=4, space="PSUM") as pspool,
        ):
            w1_t = cpool.tile([128, 32], f32, tag="w1")
            wh_t = cpool.tile([128, N_HID * 32], fp16, tag="wh")
            wo_t = cpool.tile([128, 1], fp16, tag="wo")
            wob_t = cpool.tile([128, 4], fp16, tag="wob")
            bq_t = cpool.tile([128, 20], f32, tag="bq")
            bo_t = cpool.tile([128, 1], f32, tag="bo")
            nc.sync.dma_start(out=w1_t[:], in_=w1_e[:])
            nc.sync.dma_start(out=wh_t[:], in_=wh_e[:])
            nc.sync.dma_start(out=wo_t[:], in_=wo_e[:])
            nc.sync.dma_start(out=wob_t[:], in_=wob_e[:])
            nc.sync.dma_start(out=bq_t[:], in_=bq_e[:])
            nc.sync.dma_start(out=bo_t[:], in_=bo_e[:])

            GRP = 4 if n_oct % 4 == 0 else 1  # wavefront width

            def emit_x_dma(q):
                xs = xpool.tile([128, 1024], f32, tag="xs")
                for i in range(4):
                    nc.sync.dma_start(
                        out=xs[32 * i : 32 * i + 6, :],
                        in_=x_e[6 * i : 6 * i + 6, 1024 * q : 1024 * (q + 1)],
                    )
                return xs

            def emit_layer(l, cur):
                ps = pspool.tile([128, 1024], f32, tag="ps")
                for hh in range(2):
                    for i in range(4):
                        if l == 0:
                            lhsT = w1_t[32 * i : 32 * i + 6, :]
                            rhs = cur[32 * i : 32 * i + 6, 512 * hh : 512 * hh + 512]
                        else:
                            lhsT = wh_t[32 * i : 32 * i + 32, 32 * (l - 1) : 32 * l]
                            rhs = cur[32 * i : 32 * i + 32, 512 * hh : 512 * hh + 512]
                        nc.tensor.matmul(
                            ps[32 * i : 32 * i + 32, 512 * hh : 512 * hh + 512],
                            lhsT,
                            rhs,
                            start=True,
                            stop=True,
                            tile_position=(32 * i, 32 * i),
                        )
                a_t = apool.tile([128, 1024], f32, tag="a")
                nc.scalar.activation(
                    a_t[:], ps[:], Act.Abs, bias=bq_t[:, l : l + 1], scale=1.0
                )
                h_t = hpool.tile([128, 1024], fp16, tag="h")
                nc.vector._custom_dve(
                    ssop,
                    out=h_t[:],
                    in0=ps[:],
                    in1=a_t[:],
                    s0=bq_t[:, l : l + 1],
                    s1=B_FIT,
                    imm2=A_FIT,
                )
                return h_t

            def emit_final(q, cur):
                # final layer as full-array matmuls: lhsT = h (all 4 strips,
                # K=128) over a 128-column chunk, rhs = block-diagonal Wout
                # [128, 4] -> out[m, j] = strip-j output for chunk row m.
                ps = pspool.tile([128, 32], f32, tag="ps")
                for c in range(8):
                    hh = c // 4
                    cc = c % 4
                    nc.tensor.matmul(
                        ps[:, 4 * c : 4 * c + 4],
                        cur[:, 512 * hh + 128 * cc : 512 * hh + 128 * cc + 128],
                        wob_t[:, 0:4],
                        start=(c == 0),
                        stop=(c == 7),
                        skip_group_check=True,
                    )
                ot = opool.tile([128, 32], f32, tag="ot")
                nc.scalar.activation(
                    ot[:], ps[:], Act.Identity, bias=bo_t[:, 0:1], scale=1.0
                )
                # psf col 4c+j = rows of block (hh=c//4, strip j), cols cc*128..
                for c in range(8):
                    hh = c // 4
                    cc = c % 4
                    for j in range(4):
                        blk = q * 8 + hh * 4 + j
                        r0 = blk * 512 + cc * 128
                        nc.sync.dma_start(
                            out=out_e[r0 : r0 + 128, :],
                            in_=ot[:, 4 * c + j : 4 * c + j + 1],
                        )

            assert n_oct % GRP == 0
            for base in range(0, n_oct, GRP):
                cur = [emit_x_dma(base + g) for g in range(GRP)]
                for l in range(20):
                    for g in range(GRP):
                        cur[g] = emit_layer(l, cur[g])
                for g in range(GRP):
                    emit_final(base + g, cur[g])
    nc.compile()
    return nc


def _pack_weights(W1, b1, Wh, bh, Wout, bout):
    w1q = np.zeros((128, 32), np.float32)
    whq = np.zeros((128, N_HID * 32), np.float32)
    woq = np.zeros((128, 1), np.float32)
    bq = np.zeros((128, 20), np.float32)
    boq = np.full((128, 1), np.float32(bout[0]), np.float32)
    for i in range(4):
        w1q[32 * i : 32 * i + 6, :] = W1
        for l in range(N_HID):
            whq[32 * i : 32 * i + 32, 32 * l : 32 * (l + 1)] = Wh[l]
        woq[32 * i : 32 * i + 32, 0:1] = Wout
        bq[32 * i : 32 * i + 32, 0] = b1
        bq[32 * i : 32 * i + 32, 1:20] = bh.T
    wob = np.zeros((128, 4), np.float32)
    for j in range(4):
        wob[32 * j : 32 * j + 32, j] = Wout[:, 0]
    return {
        "w1q": w1q,
        "whq": whq.astype(np.float16),
        "woq": woq.astype(np.float16),
        "wob": wob.astype(np.float16),
        "bq": bq,
        "boq": boq,
    }


def _install_ntff_hook():
    """The agent image's antenv lacks axon_hooks; shim it so trace=True works."""
    import types

    if "antenv.axon_hooks" not in sys.modules:
        mod = types.ModuleType("antenv.axon_hooks")
        state = {"hook": None}
        try:
            from trn_agent_boot.trn_boot import _ntff_profile_via_ctypes

            state["hook"] = _ntff_profile_via_ctypes("/opt/axon/libaxon_pjrt.so")
        except Exception:
            pass
        mod.get_axon_ntff_profile_hook = lambda: state["hook"]
        mod.set_axon_ntff_profile_hook = lambda h: state.__setitem__("hook", h)
        sys.modules["antenv.axon_hooks"] = mod
    from concourse import bass_utils as bu

    if not getattr(bu.upload_artifacts, "_actnet_safe", False):
        _orig = bu.upload_artifacts

        def _safe(tmpdir):
            try:
                return _orig(tmpdir)
            except Exception:
                return "local:" + tmpdir

        _safe._actnet_safe = True
        bu.upload_artifacts = _safe


def kernel(x, W1, b1, Wh, bh, Wout, bout):
    global LAST_RESULT
    from concourse.bass_utils import run_bass_kernel_spmd

    x = np.asarray(x, np.float32)
    B = x.shape[0]
    assert B % N_CORES == 0
    shard = B // N_CORES
    # pack x into the SBUF image layout: [24, shard/4] per core, where row
    # 6*i+f holds feature f of the blocks on partition-strip i
    x5 = x.reshape(N_CORES, shard // OCT_ROWS, 2, 4, NBLK, 6)  # c,q,h,i,n,f
    xq = np.ascontiguousarray(x5.transpose(0, 3, 5, 1, 2, 4)).reshape(
        N_CORES, 24, shard // 4
    )

    if ("nc", shard) not in _cache:
        _cache[("nc", shard)] = _build(shard)
    nc = _cache[("nc", shard)]

    wpack = _pack_weights(
        np.asarray(W1, np.float32),
        np.asarray(b1, np.float32),
        np.asarray(Wh, np.float32),
        np.asarray(bh, np.float32),
        np.asarray(Wout, np.float32),
        np.asarray(bout, np.float32),
    )
    in_maps = [{"xq": xq[c], **wpack} for c in range(N_CORES)]
    trace = bool(os.environ.get("ACTNET_TRACE"))
    if trace:
        _install_ntff_hook()
    res = run_bass_kernel_spmd(
        nc, in_maps, list(range(N_CORES)), trace=trace
    )
    LAST_RESULT = res
    out = np.concatenate([res.results[c]["out"] for c in range(N_CORES)], axis=0)
    return out.astype(np.float32)


if __name__ == "__main__":
    # smoke test with random data
    rng = np.random.default_rng(0)
    B = B_FULL
    inputs = dict(
        x=rng.standard_normal((B, 6), dtype=np.float32),
        W1=(rng.standard_normal((6, 32)) / np.sqrt(6)).astype(np.float32),
        b1=(rng.standard_normal(32) * 0.01).astype(np.float32),
        Wh=(rng.standard_normal((19, 32, 32)) / np.sqrt(32)).astype(np.float32),
        bh=(rng.standard_normal((19, 32)) * 0.01).astype(np.float32),
        Wout=(rng.standard_normal((32, 1)) / np.sqrt(32)).astype(np.float32),
        bout=(rng.standard_normal(1) * 0.01).astype(np.float32),
    )
    y = kernel(**inputs)
    print("kernel out", y.shape, y.dtype, y[:4, 0])


# revision 22
# speedup vs baseline: 1.6613x; 1.6613x over previous
"""Trainium2 Bass kernel for nn_ActuatorNet (20-layer tiny MLP, softsign).

Strategy (pure data parallel, 8 cores, batch 1048576 -> 131072 rows/core):
  - Activations kept TRANSPOSED in SBUF: features on partitions, rows on free
    dim.  4 partition strips (32 each) process 4 independent row-blocks
    concurrently on the PE's diagonal 32x32 tiles (tile_position=(32i,32i)).
  - Each "oct" = 8 blocks of 512 rows = [128, 1024] tiles (2 PSUM banks).
  - Per layer: 8 matmuls (bf16, fp32 psum) -> ScalarE computes |z+b| (Abs with
    per-partition bias in the free affine) -> one fused custom-DVE op computes
    softsign: y = (z+b) * recip(1+|z+b|) via the BITWISE_NOT reciprocal seed
    plus a linear minimax refinement, writing bf16 for the next matmul.
  - Layer 1 runs in fp32 straight from the DMA'd x (K=6).
  - Final layer: M=1 matmuls + ScalarE Identity(+bout), DMA out.
"""

import os
import re
import sys

import numpy as np

sys.path.insert(0, "/opt/trn_rl_repo")

N_CORES = 8
B_FULL = 1048576
SHARD = B_FULL // N_CORES  # 131072
NBLK = 512                 # rows per block = one psum bank of fp32
OCT_ROWS = 8 * NBLK        # 4096 rows per oct tile [128, 1024]
N_HID = 19

# minimax fit of 1/t ~ A + B*t over t = d*bitcast(~d) in [-4.5, -4]
A_FIT = float(np.float32(-0.4714035350548651))
B_FIT = float(np.float32(-0.05545919627798768))

SOFTSIGN_OP_NAME = "SOFTSIGN_ANT_ACTNET"

LAST_RESULT = None  # BassKernelResults of the most recent run (for test.py)

_cache = {}


def _register_softsign_op():
    """Fused softsign: out = (Src0+C0) * ~d * (C2 + C1 * d*~d), d = Src1+1.

    Src0 = z (psum fp32), Src1 = |z+b| (from ScalarE), C0 = bias [P,1] AP.
    Exactly 8 ALU stages.
    """
    from concourse import dve_ops
    from concourse.dve_spec import AluOp, Bin, C0, C1, C2, One, Spec, Src0, Src1

    if SOFTSIGN_OP_NAME in dve_ops.CUSTOM_DVE_SPECS:
        return next(o for o in dve_ops.OPS if o.name == SOFTSIGN_OP_NAME)

    _zb = Src0 + C0
    _d = Src1 + One
    _nd = Bin(AluOp.BITWISE_NOT, _d, _d)
    _t = _d * _nd
    _s = C2 + _t * C1
    _p = _zb * _nd
    body = _p * _s

    def _ref(in0, in1, s0, s1, imm2):
        zb = in0.astype(np.float32) + np.asarray(s0, np.float32)
        d = (in1.astype(np.float32) + np.float32(1.0)).astype(np.float32)
        nd = (~d.view(np.int32)).view(np.float32)
        t = (d * nd).astype(np.float32)
        s = np.float32(imm2) + t * np.asarray(s1, np.float32)
        return (zb * nd) * s

    spec = Spec(body=body, reference=_ref)
    op = dve_ops.DveOp(SOFTSIGN_OP_NAME, spec, subdim=False, uops_sha={})
    dve_ops._SUB_OPCODE_FOR_NAME[SOFTSIGN_OP_NAME] = (
        max(dve_ops._SUB_OPCODE_FOR_NAME.values()) + 1
    )
    assert dve_ops._SUB_OPCODE_FOR_NAME[SOFTSIGN_OP_NAME] < 0x20
    dve_ops.OPS.append(op)
    dve_ops.CUSTOM_DVE_SPECS[SOFTSIGN_OP_NAME] = spec
    # self-pin the uops sha (computed from our own lower() output)
    for ver in ("v3", "v4"):
        try:
            op.compile(ver)
        except ValueError as e:
            m = re.search(rf"{ver}: ([0-9a-f]{{16}})", str(e))
            if not m:
                raise
            op.uops_sha[ver] = m.group(1)
            op.compile(ver)
    return op


def _build(shard_rows):
    from concourse import bacc, mybir, tile

    ssop = _register_softsign_op()

    f32 = mybir.dt.float32
    fp16 = mybir.dt.float16
    Act = mybir.ActivationFunctionType

    assert shard_rows % OCT_ROWS == 0
    n_oct = shard_rows // OCT_ROWS

    nc = bacc.Bacc()
    x_e = nc.declare_dram_parameter("xq", [24, shard_rows // 4], f32, isOutput=False)
    w1_e = nc.declare_dram_parameter("w1q", [128, 32], f32, isOutput=False)
    wh_e = nc.declare_dram_parameter("whq", [128, N_HID * 32], fp16, isOutput=False)
    wo_e = nc.declare_dram_parameter("woq", [128, 1], fp16, isOutput=False)
    wob_e = nc.declare_dram_parameter("wob", [128, 4], fp16, isOutput=False)
    id_e = nc.declare_dram_parameter("idm", [128, 128], f32, isOutput=False)
    bq_e = nc.declare_dram_parameter("bq", [128, 20], f32, isOutput=False)
    bo_e = nc.declare_dram_parameter("boq", [128, 1], f32, isOutput=False)
    out_e = nc.declare_dram_parameter("out", [shard_rows, 1], f32, isOutput=True)

    with tile.TileContext(nc) as tc:
        with (
            tc.tile_pool(name="const", bufs=1) as cpool,
            tc.tile_pool(name="xs", bufs=5) as xpool,
            tc.tile_pool(name="h", bufs=9) as hpool,
            tc.tile_pool(name="a", bufs=5) as apool,
            tc.tile_pool(name="ot", bufs=4) as opool,
            tc.tile_pool(name="ps", bufs=4, space="PSUM") as pspool,
        ):
            w1_t = cpool.tile([128, 32], f32, tag="w1")
            wh_t = cpool.tile([128, N_HID * 32], fp16, tag="wh")
            wo_t = cpool.tile([128, 1], fp16, tag="wo")
            wob_t = cpool.tile([128, 4], fp16, tag="wob")
            id_t = cpool.tile([128, 128], f32, tag="idm")
            bq_t = cpool.tile([128, 20], f32, tag="bq")
            bo_t = cpool.tile([128, 1], f32, tag="bo")
            nc.sync.dma_start(out=w1_t[:], in_=w1_e[:])
            nc.sync.dma_start(out=wh_t[:], in_=wh_e[:])
            nc.sync.dma_start(out=wo_t[:], in_=wo_e[:])
            nc.sync.dma_start(out=wob_t[:], in_=wob_e[:])
            nc.sync.dma_start(out=id_t[:], in_=id_e[:])
            nc.sync.dma_start(out=bq_t[:], in_=bq_e[:])
            nc.sync.dma_start(out=bo_t[:], in_=bo_e[:])

            GRP = 4 if n_oct % 4 == 0 else 1  # wavefront width

            def emit_x_dma(q):
                xs = xpool.tile([128, 1024], f32, tag="xs")
                for i in range(4):
                    nc.sync.dma_start(
                        out=xs[32 * i : 32 * i + 6, :],
                        in_=x_e[6 * i : 6 * i + 6, 1024 * q : 1024 * (q + 1)],
                    )
                return xs

            def emit_layer(l, cur):
                ps = pspool.tile([128, 1024], f32, tag="ps")
                for hh in range(2):
                    for i in range(4):
                        if l == 0:
                            lhsT = w1_t[32 * i : 32 * i + 6, :]
                            rhs = cur[32 * i : 32 * i + 6, 512 * hh : 512 * hh + 512]
                        else:
                            lhsT = wh_t[32 * i : 32 * i + 32, 32 * (l - 1) : 32 * l]
                            rhs = cur[32 * i : 32 * i + 32, 512 * hh : 512 * hh + 512]
                        nc.tensor.matmul(
                            ps[32 * i : 32 * i + 32, 512 * hh : 512 * hh + 512],
                            lhsT,
                            rhs,
                            start=True,
                            stop=True,
                            tile_position=(32 * i, 32 * i),
                        )
                a_t = apool.tile([128, 1024], f32, tag="a")
                nc.scalar.activation(
                    a_t[:], ps[:], Act.Abs, bias=bq_t[:, l : l + 1], scale=1.0
                )
                h_t = hpool.tile([128, 1024], fp16, tag="h")
                nc.vector._custom_dve(
                    ssop,
                    out=h_t[:],
                    in0=ps[:],
                    in1=a_t[:],
                    s0=bq_t[:, l : l + 1],
                    s1=B_FIT,
                    imm2=A_FIT,
                )
                return h_t

            def emit_final(q, cur):
                # final layer as full-array matmuls: lhsT = h (all 4 strips,
                # K=128) over a 128-column chunk, rhs = block-diagonal Wout
                # [128, 4] -> out[m, j] = strip-j output for chunk row m.
                ps = pspool.tile([128, 32], f32, tag="ps")
                for c in range(8):
                    hh = c // 4
                    cc = c % 4
                    nc.tensor.matmul(
                        ps[:, 16 * (c // 4) + (c % 4) : 16 * (c // 4) + (c % 4) + 13 : 4],
                        cur[:, 512 * hh + 128 * cc : 512 * hh + 128 * cc + 128],
                        wob_t[:, 0:4],
                        start=(c == 0),
                        stop=(c == 7),
                        skip_group_check=True,
                    )
                ot = opool.tile([128, 32], f32, tag="ot")
                nc.scalar.activation(
                    ot[:], ps[:], Act.Identity, bias=bo_t[:, 0:1], scale=1.0
                )
                # PE-transpose so output rows sit on the free dim, then one
                # dense DMA (512B runs) per oct
                pst = pspool.tile([128, 128], f32, tag="ps")
                nc.tensor.transpose(pst[0:32, 0:128], ot[:, 0:32], id_t[:, 0:128])
                ot2 = opool.tile([32, 128], f32, tag="ot2")
                nc.vector.tensor_copy(ot2[:], pst[0:32, 0:128])
                nc.sync.dma_start(
                    out=out_e[q * 4096 : (q + 1) * 4096, :].rearrange(
                        "(k p) o -> k (p o)", p=128
                    ),
                    in_=ot2[:],
                )

            assert n_oct % GRP == 0
            for base in range(0, n_oct, GRP):
                cur = [emit_x_dma(base + g) for g in range(GRP)]
                for l in range(20):
                    for g in range(GRP):
                        cur[g] = emit_layer(l, cur[g])
                for g in range(GRP):
                    emit_final(base + g, cur[g])
    nc.compile()
    return nc


def _pack_weights(W1, b1, Wh, bh, Wout, bout):
    w1q = np.zeros((128, 32), np.float32)
    whq = np.zeros((128, N_HID * 32), np.float32)
    woq = np.zeros((128, 1), np.float32)
    bq = np.zeros((128, 20), np.float32)
    boq = np.full((128, 1), np.float32(bout[0]), np.float32)
    for i in range(4):
        w1q[32 * i : 32 * i + 6, :] = W1
        for l in range(N_HID):
            whq[32 * i : 32 * i + 32, 32 * l : 32 * (l + 1)] = Wh[l]
        woq[32 * i : 32 * i + 32, 0:1] = Wout
        bq[32 * i : 32 * i + 32, 0] = b1
        bq[32 * i : 32 * i + 32, 1:20] = bh.T
    wob = np.zeros((128, 4), np.float32)
    for j in range(4):
        wob[32 * j : 32 * j + 32, j] = Wout[:, 0]
    return {
        "w1q": w1q,
        "whq": whq.astype(np.float16),
        "woq": woq.astype(np.float16),
        "wob": wob.astype(np.float16),
        "idm": np.eye(128, dtype=np.float32),
        "bq": bq,
        "boq": boq,
    }


def _install_ntff_hook():
    """The agent image's antenv lacks axon_hooks; shim it so trace=True works."""
    import types

    if "antenv.axon_hooks" not in sys.modules:
        mod = types.ModuleType("antenv.axon_hooks")
        state = {"hook": None}
        try:
            from trn_agent_boot.trn_boot import _ntff_profile_via_ctypes

            state["hook"] = _ntff_profile_via_ctypes("/opt/axon/libaxon_pjrt.so")
        except Exception:
            pass
        mod.get_axon_ntff_profile_hook = lambda: state["hook"]
        mod.set_axon_ntff_profile_hook = lambda h: state.__setitem__("hook", h)
        sys.modules["antenv.axon_hooks"] = mod
    from concourse import bass_utils as bu

    if not getattr(bu.upload_artifacts, "_actnet_safe", False):
        _orig = bu.upload_artifacts

        def _safe(tmpdir):
            try:
                return _orig(tmpdir)
            except Exception:
                return "local:" + tmpdir

        _safe._actnet_safe = True
        bu.upload_artifacts = _safe


def kernel(x, W1, b1, Wh, bh, Wout, bout):
    global LAST_RESULT
    from concourse.bass_utils import run_bass_kernel_spmd

    x = np.asarray(x, np.float32)
    B = x.shape[0]
    assert B % N_CORES == 0
    shard = B // N_CORES
    # pack x into the SBUF image layout: [24, shard/4] per core, where row
    # 6*i+f holds feature f of the blocks on partition-strip i
    x5 = x.reshape(N_CORES, shard // OCT_ROWS, 2, 4, NBLK, 6)  # c,q,h,i,n,f
    xq = np.ascontiguousarray(x5.transpose(0, 3, 5, 1, 2, 4)).reshape(
        N_CORES, 24, shard // 4
    )

    if ("nc", shard) not in _cache:
        _cache[("nc", shard)] = _build(shard)
    nc = _cache[("nc", shard)]

    wpack = _pack_weights(
        np.asarray(W1, np.float32),
        np.asarray(b1, np.float32),
        np.asarray(Wh, np.float32),
        np.asarray(bh, np.float32),
        np.asarray(Wout, np.float32),
        np.asarray(bout, np.float32),
    )
    in_maps = [{"xq": xq[c], **wpack} for c in range(N_CORES)]
    trace = bool(os.environ.get("ACTNET_TRACE"))
    if trace:
        _install_ntff_hook()
    res = run_bass_kernel_spmd(
        nc, in_maps, list(range(N_CORES)), trace=trace
    )
    LAST_RESULT = res
    out = np.concatenate([res.results[c]["out"] for c in range(N_CORES)], axis=0)
    return out.astype(np.float32)


if __name__ == "__main__":
    # smoke test with random data
    rng = np.random.default_rng(0)
    B = B_FULL
    inputs = dict(
        x=rng.standard_normal((B, 6), dtype=np.float32),
        W1=(rng.standard_normal((6, 32)) / np.sqrt(6)).astype(np.float32),
        b1=(rng.standard_normal(32) * 0.01).astype(np.float32),
        Wh=(rng.standard_normal((19, 32, 32)) / np.sqrt(32)).astype(np.float32),
        bh=(rng.standard_normal((19, 32)) * 0.01).astype(np.float32),
        Wout=(rng.standard_normal((32, 1)) / np.sqrt(32)).astype(np.float32),
        bout=(rng.standard_normal(1) * 0.01).astype(np.float32),
    )
    y = kernel(**inputs)
    print("kernel out", y.shape, y.dtype, y[:4, 0])


# revision 23
# speedup vs baseline: 1.7200x; 1.0353x over previous
"""Trainium2 Bass kernel for nn_ActuatorNet (20-layer tiny MLP, softsign).

Strategy (pure data parallel, 8 cores, batch 1048576 -> 131072 rows/core):
  - Activations kept TRANSPOSED in SBUF: features on partitions, rows on free
    dim.  4 partition strips (32 each) process 4 independent row-blocks
    concurrently on the PE's diagonal 32x32 tiles (tile_position=(32i,32i)).
  - Each "oct" = 8 blocks of 512 rows = [128, 1024] tiles (2 PSUM banks).
  - Per layer: 8 matmuls (bf16, fp32 psum) -> ScalarE computes |z+b| (Abs with
    per-partition bias in the free affine) -> one fused custom-DVE op computes
    softsign: y = (z+b) * recip(1+|z+b|) via the BITWISE_NOT reciprocal seed
    plus a linear minimax refinement, writing bf16 for the next matmul.
  - Layer 1 runs in fp32 straight from the DMA'd x (K=6).
  - Final layer: M=1 matmuls + ScalarE Identity(+bout), DMA out.
"""

import os
import re
import sys

import numpy as np

sys.path.insert(0, "/opt/trn_rl_repo")

N_CORES = 8
B_FULL = 1048576
SHARD = B_FULL // N_CORES  # 131072
NBLK = 512                 # rows per block = one psum bank of fp32
OCT_ROWS = 8 * NBLK        # 4096 rows per oct tile [128, 1024]
N_HID = 19

# minimax fit of 1/t ~ A + B*t over t = d*bitcast(~d) in [-4.5, -4]
A_FIT = float(np.float32(-0.4714035350548651))
B_FIT = float(np.float32(-0.05545919627798768))

SOFTSIGN_OP_NAME = "SOFTSIGN_ANT_ACTNET"

LAST_RESULT = None  # BassKernelResults of the most recent run (for test.py)

_cache = {}


def _register_softsign_op():
    """Fused softsign: out = (Src0+C0) * ~d * (C2 + C1 * d*~d), d = Src1+1.

    Src0 = z (psum fp32), Src1 = |z+b| (from ScalarE), C0 = bias [P,1] AP.
    Exactly 8 ALU stages.
    """
    from concourse import dve_ops
    from concourse.dve_spec import AluOp, Bin, C0, C1, C2, One, Spec, Src0, Src1

    if SOFTSIGN_OP_NAME in dve_ops.CUSTOM_DVE_SPECS:
        return next(o for o in dve_ops.OPS if o.name == SOFTSIGN_OP_NAME)

    _zb = Src0 + C0
    _d = Src1 + One
    _nd = Bin(AluOp.BITWISE_NOT, _d, _d)
    _t = _d * _nd
    _s = C2 + _t * C1
    _p = _zb * _nd
    body = _p * _s

    def _ref(in0, in1, s0, s1, imm2):
        zb = in0.astype(np.float32) + np.asarray(s0, np.float32)
        d = (in1.astype(np.float32) + np.float32(1.0)).astype(np.float32)
        nd = (~d.view(np.int32)).view(np.float32)
        t = (d * nd).astype(np.float32)
        s = np.float32(imm2) + t * np.asarray(s1, np.float32)
        return (zb * nd) * s

    spec = Spec(body=body, reference=_ref)
    op = dve_ops.DveOp(SOFTSIGN_OP_NAME, spec, subdim=False, uops_sha={})
    dve_ops._SUB_OPCODE_FOR_NAME[SOFTSIGN_OP_NAME] = (
        max(dve_ops._SUB_OPCODE_FOR_NAME.values()) + 1
    )
    assert dve_ops._SUB_OPCODE_FOR_NAME[SOFTSIGN_OP_NAME] < 0x20
    dve_ops.OPS.append(op)
    dve_ops.CUSTOM_DVE_SPECS[SOFTSIGN_OP_NAME] = spec
    # self-pin the uops sha (computed from our own lower() output)
    for ver in ("v3", "v4"):
        try:
            op.compile(ver)
        except ValueError as e:
            m = re.search(rf"{ver}: ([0-9a-f]{{16}})", str(e))
            if not m:
                raise
            op.uops_sha[ver] = m.group(1)
            op.compile(ver)
    return op


def _build(shard_rows):
    from concourse import bacc, mybir, tile

    ssop = _register_softsign_op()

    f32 = mybir.dt.float32
    fp16 = mybir.dt.float16
    Act = mybir.ActivationFunctionType

    assert shard_rows % OCT_ROWS == 0
    n_oct = shard_rows // OCT_ROWS

    nc = bacc.Bacc()
    x_e = nc.declare_dram_parameter("xq", [24, shard_rows // 4], f32, isOutput=False)
    w1_e = nc.declare_dram_parameter("w1q", [128, 32], f32, isOutput=False)
    wh_e = nc.declare_dram_parameter("whq", [128, N_HID * 32], fp16, isOutput=False)
    wo_e = nc.declare_dram_parameter("woq", [128, 1], fp16, isOutput=False)
    wob_e = nc.declare_dram_parameter("wob", [128, 4], fp16, isOutput=False)
    id_e = nc.declare_dram_parameter("idm", [128, 128], f32, isOutput=False)
    bq_e = nc.declare_dram_parameter("bq", [128, 20], f32, isOutput=False)
    bo_e = nc.declare_dram_parameter("boq", [128, 1], f32, isOutput=False)
    out_e = nc.declare_dram_parameter("out", [shard_rows, 1], f32, isOutput=True)

    with tile.TileContext(nc) as tc:
        with (
            tc.tile_pool(name="const", bufs=1) as cpool,
            tc.tile_pool(name="xs", bufs=5) as xpool,
            tc.tile_pool(name="h", bufs=13) as hpool,
            tc.tile_pool(name="a", bufs=5) as apool,
            tc.tile_pool(name="ot", bufs=4) as opool,
            tc.tile_pool(name="ps", bufs=4, space="PSUM") as pspool,
        ):
            w1_t = cpool.tile([128, 32], f32, tag="w1")
            wh_t = cpool.tile([128, N_HID * 32], fp16, tag="wh")
            wo_t = cpool.tile([128, 1], fp16, tag="wo")
            wob_t = cpool.tile([128, 4], fp16, tag="wob")
            id_t = cpool.tile([128, 128], f32, tag="idm")
            bq_t = cpool.tile([128, 20], f32, tag="bq")
            bo_t = cpool.tile([128, 1], f32, tag="bo")
            nc.sync.dma_start(out=w1_t[:], in_=w1_e[:])
            nc.sync.dma_start(out=wh_t[:], in_=wh_e[:])
            nc.sync.dma_start(out=wo_t[:], in_=wo_e[:])
            nc.sync.dma_start(out=wob_t[:], in_=wob_e[:])
            nc.sync.dma_start(out=id_t[:], in_=id_e[:])
            nc.sync.dma_start(out=bq_t[:], in_=bq_e[:])
            nc.sync.dma_start(out=bo_t[:], in_=bo_e[:])

            GRP = 4 if n_oct % 4 == 0 else 1  # wavefront width

            def emit_x_dma(q):
                xs = xpool.tile([128, 1024], f32, tag="xs")
                for i in range(4):
                    nc.sync.dma_start(
                        out=xs[32 * i : 32 * i + 6, :],
                        in_=x_e[6 * i : 6 * i + 6, 1024 * q : 1024 * (q + 1)],
                    )
                return xs

            def emit_layer(l, cur):
                ps = pspool.tile([128, 1024], f32, tag="ps")
                for hh in range(2):
                    for i in range(4):
                        if l == 0:
                            lhsT = w1_t[32 * i : 32 * i + 6, :]
                            rhs = cur[32 * i : 32 * i + 6, 512 * hh : 512 * hh + 512]
                        else:
                            lhsT = wh_t[32 * i : 32 * i + 32, 32 * (l - 1) : 32 * l]
                            rhs = cur[32 * i : 32 * i + 32, 512 * hh : 512 * hh + 512]
                        nc.tensor.matmul(
                            ps[32 * i : 32 * i + 32, 512 * hh : 512 * hh + 512],
                            lhsT,
                            rhs,
                            start=True,
                            stop=True,
                            tile_position=(32 * i, 32 * i),
                        )
                a_t = apool.tile([128, 1024], f32, tag="a")
                nc.scalar.activation(
                    a_t[:], ps[:], Act.Abs, bias=bq_t[:, l : l + 1], scale=1.0
                )
                h_t = hpool.tile([128, 1024], fp16, tag="h")
                nc.vector._custom_dve(
                    ssop,
                    out=h_t[:],
                    in0=ps[:],
                    in1=a_t[:],
                    s0=bq_t[:, l : l + 1],
                    s1=B_FIT,
                    imm2=A_FIT,
                )
                return h_t

            def emit_final(q, cur):
                # final layer as full-array matmuls: lhsT = h (all 4 strips,
                # K=128) over a 128-column chunk, rhs = block-diagonal Wout
                # [128, 4] -> out[m, j] = strip-j output for chunk row m.
                ps = pspool.tile([128, 1024], f32, tag="ps")
                for c in range(8):
                    hh = c // 4
                    cc = c % 4
                    nc.tensor.matmul(
                        ps[:, 16 * (c // 4) + (c % 4) : 16 * (c // 4) + (c % 4) + 13 : 4],
                        cur[:, 512 * hh + 128 * cc : 512 * hh + 128 * cc + 128],
                        wob_t[:, 0:4],
                        start=(c == 0),
                        stop=(c == 7),
                        skip_group_check=True,
                    )
                ot = opool.tile([128, 32], f32, tag="ot")
                nc.scalar.activation(
                    ot[:], ps[:, 0:32], Act.Identity, bias=bo_t[:, 0:1], scale=1.0
                )
                # PE-transpose so output rows sit on the free dim, then one
                # dense DMA (512B runs) per oct
                nc.tensor.transpose(ps[0:32, 512:640], ot[:, 0:32], id_t[:, 0:128])
                ot2 = opool.tile([32, 128], f32, tag="ot2")
                nc.scalar.copy(ot2[:], ps[0:32, 512:640])
                nc.sync.dma_start(
                    out=out_e[q * 4096 : (q + 1) * 4096, :].rearrange(
                        "(k p) o -> k (p o)", p=128
                    ),
                    in_=ot2[:],
                )

            assert n_oct % GRP == 0
            prev = None  # (base, cur list) of the previous group, finals pending
            for base in range(0, n_oct, GRP):
                cur = [emit_x_dma(base + g) for g in range(GRP)]
                for l in range(20):
                    for g in range(GRP):
                        cur[g] = emit_layer(l, cur[g])
                    if prev is not None and 1 <= l <= GRP:
                        emit_final(prev[0] + (l - 1), prev[1][l - 1])
                        if l == GRP:
                            prev = None
                prev = (base, list(cur))
            for g in range(GRP):
                emit_final(prev[0] + g, prev[1][g])
    nc.compile()
    return nc


def _pack_weights(W1, b1, Wh, bh, Wout, bout):
    w1q = np.zeros((128, 32), np.float32)
    whq = np.zeros((128, N_HID * 32), np.float32)
    woq = np.zeros((128, 1), np.float32)
    bq = np.zeros((128, 20), np.float32)
    boq = np.full((128, 1), np.float32(bout[0]), np.float32)
    for i in range(4):
        w1q[32 * i : 32 * i + 6, :] = W1
        for l in range(N_HID):
            whq[32 * i : 32 * i + 32, 32 * l : 32 * (l + 1)] = Wh[l]
        woq[32 * i : 32 * i + 32, 0:1] = Wout
        bq[32 * i : 32 * i + 32, 0] = b1
        bq[32 * i : 32 * i + 32, 1:20] = bh.T
    wob = np.zeros((128, 4), np.float32)
    for j in range(4):
        wob[32 * j : 32 * j + 32, j] = Wout[:, 0]
    return {
        "w1q": w1q,
        "whq": whq.astype(np.float16),
        "woq": woq.astype(np.float16),
        "wob": wob.astype(np.float16),
        "idm": np.eye(128, dtype=np.float32),
        "bq": bq,
        "boq": boq,
    }


def _install_ntff_hook():
    """The agent image's antenv lacks axon_hooks; shim it so trace=True works."""
    import types

    if "antenv.axon_hooks" not in sys.modules:
        mod = types.ModuleType("antenv.axon_hooks")
        state = {"hook": None}
        try:
            from trn_agent_boot.trn_boot import _ntff_profile_via_ctypes

            state["hook"] = _ntff_profile_via_ctypes("/opt/axon/libaxon_pjrt.so")
        except Exception:
            pass
        mod.get_axon_ntff_profile_hook = lambda: state["hook"]
        mod.set_axon_ntff_profile_hook = lambda h: state.__setitem__("hook", h)
        sys.modules["antenv.axon_hooks"] = mod
    from concourse import bass_utils as bu

    if not getattr(bu.upload_artifacts, "_actnet_safe", False):
        _orig = bu.upload_artifacts

        def _safe(tmpdir):
            try:
                return _orig(tmpdir)
            except Exception:
                return "local:" + tmpdir

        _safe._actnet_safe = True
        bu.upload_artifacts = _safe


def kernel(x, W1, b1, Wh, bh, Wout, bout):
    global LAST_RESULT
    from concourse.bass_utils import run_bass_kernel_spmd

    x = np.asarray(x, np.float32)
    B = x.shape[0]
    assert B % N_CORES == 0
    shard = B // N_CORES
    # pack x into the SBUF image layout: [24, shard/4] per core, where row
    # 6*i+f holds feature f of the blocks on partition-strip i
    x5 = x.reshape(N_CORES, shard // OCT_ROWS, 2, 4, NBLK, 6)  # c,q,h,i,n,f
    xq = np.ascontiguousarray(x5.transpose(0, 3, 5, 1, 2, 4)).reshape(
        N_CORES, 24, shard // 4
    )

    if ("nc", shard) not in _cache:
        _cache[("nc", shard)] = _build(shard)
    nc = _cache[("nc", shard)]

    wpack = _pack_weights(
        np.asarray(W1, np.float32),
        np.asarray(b1, np.float32),
        np.asarray(Wh, np.float32),
        np.asarray(bh, np.float32),
        np.asarray(Wout, np.float32),
        np.asarray(bout, np.float32),
    )
    in_maps = [{"xq": xq[c], **wpack} for c in range(N_CORES)]
    trace = bool(os.environ.get("ACTNET_TRACE"))
    if trace:
        _install_ntff_hook()
    res = run_bass_kernel_spmd(
        nc, in_maps, list(range(N_CORES)), trace=trace
    )
    LAST_RESULT = res
    out = np.concatenate([res.results[c]["out"] for c in range(N_CORES)], axis=0)
    return out.astype(np.float32)


if __name__ == "__main__":
    # smoke test with random data
    rng = np.random.default_rng(0)
    B = B_FULL
    inputs = dict(
        x=rng.standard_normal((B, 6), dtype=np.float32),
        W1=(rng.standard_normal((6, 32)) / np.sqrt(6)).astype(np.float32),
        b1=(rng.standard_normal(32) * 0.01).astype(np.float32),
        Wh=(rng.standard_normal((19, 32, 32)) / np.sqrt(32)).astype(np.float32),
        bh=(rng.standard_normal((19, 32)) * 0.01).astype(np.float32),
        Wout=(rng.standard_normal((32, 1)) / np.sqrt(32)).astype(np.float32),
        bout=(rng.standard_normal(1) * 0.01).astype(np.float32),
    )
    y = kernel(**inputs)
    print("kernel out", y.shape, y.dtype, y[:4, 0])


# revision 24
# speedup vs baseline: 1.7250x; 1.0029x over previous
"""Trainium2 Bass kernel for nn_ActuatorNet (20-layer tiny MLP, softsign).

Strategy (pure data parallel, 8 cores, batch 1048576 -> 131072 rows/core):
  - Activations kept TRANSPOSED in SBUF: features on partitions, rows on free
    dim.  4 partition strips (32 each) process 4 independent row-blocks
    concurrently on the PE's diagonal 32x32 tiles (tile_position=(32i,32i)).
  - Each "oct" = 8 blocks of 512 rows = [128, 1024] tiles (2 PSUM banks).
  - Per layer: 8 fp16 matmuls (fp32 psum) -> ScalarE computes |z+b| (Abs with
    per-partition bias in the free affine) -> one fused 8-stage custom-DVE op
    computes softsign y = (z+b) * recip(1+|z+b|) via the BITWISE_NOT
    reciprocal seed plus a linear minimax refinement (max rel err 1.7e-3),
    writing fp16 for the next matmul.  DVE is the bottleneck engine
    (1 elem/cycle/lane, ~84M activation elements per core).
  - Octs advance through the layers in a software-pipelined wavefront of 4 so
    PE/ACT/DVE overlap across octs; psum tiles rotate through all 8 banks.
  - Layer 1 runs in fp32 straight from the DMA'd (host-transposed) x (K=6).
  - Final layer: full-array K=128 matmuls against a block-diagonal Wout
    (N=4 per 128-row chunk), drained via PE-transpose so the output DMA is
    dense; finals are deferred into the next group's early layers.
"""

import os
import re
import sys

import numpy as np

sys.path.insert(0, "/opt/trn_rl_repo")

N_CORES = 8
B_FULL = 1048576
SHARD = B_FULL // N_CORES  # 131072
NBLK = 512                 # rows per block = one psum bank of fp32
OCT_ROWS = 8 * NBLK        # 4096 rows per oct tile [128, 1024]
N_HID = 19

# minimax fit of 1/t ~ A + B*t over t = d*bitcast(~d) in [-4.5, -4]
A_FIT = float(np.float32(-0.4714035350548651))
B_FIT = float(np.float32(-0.05545919627798768))

SOFTSIGN_OP_NAME = "SOFTSIGN_ANT_ACTNET"

LAST_RESULT = None  # BassKernelResults of the most recent run (for test.py)

_cache = {}


def _register_softsign_op():
    """Fused softsign: out = (Src0+C0) * ~d * (C2 + C1 * d*~d), d = Src1+1.

    Src0 = z (psum fp32), Src1 = |z+b| (from ScalarE), C0 = bias [P,1] AP.
    Exactly 8 ALU stages.
    """
    from concourse import dve_ops
    from concourse.dve_spec import AluOp, Bin, C0, C1, C2, One, Spec, Src0, Src1

    if SOFTSIGN_OP_NAME in dve_ops.CUSTOM_DVE_SPECS:
        return next(o for o in dve_ops.OPS if o.name == SOFTSIGN_OP_NAME)

    _zb = Src0 + C0
    _d = Src1 + One
    _nd = Bin(AluOp.BITWISE_NOT, _d, _d)
    _t = _d * _nd
    _s = C2 + _t * C1
    _p = _zb * _nd
    body = _p * _s

    def _ref(in0, in1, s0, s1, imm2):
        zb = in0.astype(np.float32) + np.asarray(s0, np.float32)
        d = (in1.astype(np.float32) + np.float32(1.0)).astype(np.float32)
        nd = (~d.view(np.int32)).view(np.float32)
        t = (d * nd).astype(np.float32)
        s = np.float32(imm2) + t * np.asarray(s1, np.float32)
        return (zb * nd) * s

    spec = Spec(body=body, reference=_ref)
    op = dve_ops.DveOp(SOFTSIGN_OP_NAME, spec, subdim=False, uops_sha={})
    dve_ops._SUB_OPCODE_FOR_NAME[SOFTSIGN_OP_NAME] = (
        max(dve_ops._SUB_OPCODE_FOR_NAME.values()) + 1
    )
    assert dve_ops._SUB_OPCODE_FOR_NAME[SOFTSIGN_OP_NAME] < 0x20
    dve_ops.OPS.append(op)
    dve_ops.CUSTOM_DVE_SPECS[SOFTSIGN_OP_NAME] = spec
    # self-pin the uops sha (computed from our own lower() output)
    for ver in ("v3", "v4"):
        try:
            op.compile(ver)
        except ValueError as e:
            m = re.search(rf"{ver}: ([0-9a-f]{{16}})", str(e))
            if not m:
                raise
            op.uops_sha[ver] = m.group(1)
            op.compile(ver)
    return op


def _build(shard_rows):
    from concourse import bacc, mybir, tile

    ssop = _register_softsign_op()

    f32 = mybir.dt.float32
    fp16 = mybir.dt.float16
    Act = mybir.ActivationFunctionType

    assert shard_rows % OCT_ROWS == 0
    n_oct = shard_rows // OCT_ROWS

    nc = bacc.Bacc()
    x_e = nc.declare_dram_parameter("xq", [24, shard_rows // 4], f32, isOutput=False)
    w1_e = nc.declare_dram_parameter("w1q", [128, 32], f32, isOutput=False)
    wh_e = nc.declare_dram_parameter("whq", [128, N_HID * 32], fp16, isOutput=False)
    wo_e = nc.declare_dram_parameter("woq", [128, 1], fp16, isOutput=False)
    wob_e = nc.declare_dram_parameter("wob", [128, 4], fp16, isOutput=False)
    id_e = nc.declare_dram_parameter("idm", [128, 128], f32, isOutput=False)
    bq_e = nc.declare_dram_parameter("bq", [128, 20], f32, isOutput=False)
    bo_e = nc.declare_dram_parameter("boq", [128, 1], f32, isOutput=False)
    out_e = nc.declare_dram_parameter("out", [shard_rows, 1], f32, isOutput=True)

    with tile.TileContext(nc) as tc:
        with (
            tc.tile_pool(name="const", bufs=1) as cpool,
            tc.tile_pool(name="xs", bufs=5) as xpool,
            tc.tile_pool(name="h", bufs=13) as hpool,
            tc.tile_pool(name="a", bufs=5) as apool,
            tc.tile_pool(name="ot", bufs=4) as opool,
            tc.tile_pool(name="ps", bufs=4, space="PSUM") as pspool,
        ):
            w1_t = cpool.tile([128, 32], f32, tag="w1")
            wh_t = cpool.tile([128, N_HID * 32], fp16, tag="wh")
            wo_t = cpool.tile([128, 1], fp16, tag="wo")
            wob_t = cpool.tile([128, 4], fp16, tag="wob")
            id_t = cpool.tile([128, 128], f32, tag="idm")
            bq_t = cpool.tile([128, 20], f32, tag="bq")
            bo_t = cpool.tile([128, 1], f32, tag="bo")
            nc.sync.dma_start(out=w1_t[:], in_=w1_e[:])
            nc.sync.dma_start(out=wh_t[:], in_=wh_e[:])
            nc.sync.dma_start(out=wo_t[:], in_=wo_e[:])
            nc.sync.dma_start(out=wob_t[:], in_=wob_e[:])
            nc.sync.dma_start(out=id_t[:], in_=id_e[:])
            nc.sync.dma_start(out=bq_t[:], in_=bq_e[:])
            nc.sync.dma_start(out=bo_t[:], in_=bo_e[:])

            GRP = 4 if n_oct % 4 == 0 else 1  # wavefront width

            def emit_x_dma(q):
                xs = xpool.tile([128, 1024], f32, tag="xs")
                for i in range(4):
                    nc.sync.dma_start(
                        out=xs[32 * i : 32 * i + 6, :],
                        in_=x_e[6 * i : 6 * i + 6, 1024 * q : 1024 * (q + 1)],
                    )
                return xs

            def emit_layer(l, cur):
                ps = pspool.tile([128, 1024], f32, tag="ps")
                for hh in range(2):
                    for i in range(4):
                        if l == 0:
                            lhsT = w1_t[32 * i : 32 * i + 6, :]
                            rhs = cur[32 * i : 32 * i + 6, 512 * hh : 512 * hh + 512]
                        else:
                            lhsT = wh_t[32 * i : 32 * i + 32, 32 * (l - 1) : 32 * l]
                            rhs = cur[32 * i : 32 * i + 32, 512 * hh : 512 * hh + 512]
                        nc.tensor.matmul(
                            ps[32 * i : 32 * i + 32, 512 * hh : 512 * hh + 512],
                            lhsT,
                            rhs,
                            start=True,
                            stop=True,
                            tile_position=(32 * i, 32 * i),
                        )
                a_t = apool.tile([128, 1024], f32, tag="a")
                nc.scalar.activation(
                    a_t[:], ps[:], Act.Abs, bias=bq_t[:, l : l + 1], scale=1.0
                )
                h_t = hpool.tile([128, 1024], fp16, tag="h")
                nc.vector._custom_dve(
                    ssop,
                    out=h_t[:],
                    in0=ps[:],
                    in1=a_t[:],
                    s0=bq_t[:, l : l + 1],
                    s1=B_FIT,
                    imm2=A_FIT,
                )
                return h_t

            def emit_final(q, cur):
                # final layer as full-array matmuls: lhsT = h (all 4 strips,
                # K=128) over a 128-column chunk, rhs = block-diagonal Wout
                # [128, 4] -> out[m, j] = strip-j output for chunk row m.
                ps = pspool.tile([128, 1024], f32, tag="ps")
                for c in range(8):
                    hh = c // 4
                    cc = c % 4
                    nc.tensor.matmul(
                        ps[:, 16 * (c // 4) + (c % 4) : 16 * (c // 4) + (c % 4) + 13 : 4],
                        cur[:, 512 * hh + 128 * cc : 512 * hh + 128 * cc + 128],
                        wob_t[:, 0:4],
                        start=(c == 0),
                        stop=(c == 7),
                        skip_group_check=True,
                    )
                ot = opool.tile([128, 32], f32, tag="ot")
                nc.scalar.activation(
                    ot[:], ps[:, 0:32], Act.Identity, bias=bo_t[:, 0:1], scale=1.0
                )
                # PE-transpose so output rows sit on the free dim, then one
                # dense DMA (512B runs) per oct
                nc.tensor.transpose(ps[0:32, 512:640], ot[:, 0:32], id_t[:, 0:128])
                ot2 = opool.tile([32, 128], f32, tag="ot2")
                nc.scalar.copy(ot2[:], ps[0:32, 512:640])
                nc.sync.dma_start(
                    out=out_e[q * 4096 : (q + 1) * 4096, :].rearrange(
                        "(k p) o -> k (p o)", p=128
                    ),
                    in_=ot2[:],
                )

            assert n_oct % GRP == 0
            prev = None  # (base, cur list) of the previous group, finals pending
            for base in range(0, n_oct, GRP):
                cur = [emit_x_dma(base + g) for g in range(GRP)]
                for l in range(20):
                    for g in range(GRP):
                        cur[g] = emit_layer(l, cur[g])
                    if prev is not None and 1 <= l <= GRP:
                        emit_final(prev[0] + (l - 1), prev[1][l - 1])
                        if l == GRP:
                            prev = None
                prev = (base, list(cur))
            for g in range(GRP):
                emit_final(prev[0] + g, prev[1][g])
    nc.compile()
    return nc


def _pack_weights(W1, b1, Wh, bh, Wout, bout):
    w1q = np.zeros((128, 32), np.float32)
    whq = np.zeros((128, N_HID * 32), np.float32)
    woq = np.zeros((128, 1), np.float32)
    bq = np.zeros((128, 20), np.float32)
    boq = np.full((128, 1), np.float32(bout[0]), np.float32)
    for i in range(4):
        w1q[32 * i : 32 * i + 6, :] = W1
        for l in range(N_HID):
            whq[32 * i : 32 * i + 32, 32 * l : 32 * (l + 1)] = Wh[l]
        woq[32 * i : 32 * i + 32, 0:1] = Wout
        bq[32 * i : 32 * i + 32, 0] = b1
        bq[32 * i : 32 * i + 32, 1:20] = bh.T
    wob = np.zeros((128, 4), np.float32)
    for j in range(4):
        wob[32 * j : 32 * j + 32, j] = Wout[:, 0]
    return {
        "w1q": w1q,
        "whq": whq.astype(np.float16),
        "woq": woq.astype(np.float16),
        "wob": wob.astype(np.float16),
        "idm": np.eye(128, dtype=np.float32),
        "bq": bq,
        "boq": boq,
    }


def _install_ntff_hook():
    """The agent image's antenv lacks axon_hooks; shim it so trace=True works."""
    import types

    if "antenv.axon_hooks" not in sys.modules:
        mod = types.ModuleType("antenv.axon_hooks")
        state = {"hook": None}
        try:
            from trn_agent_boot.trn_boot import _ntff_profile_via_ctypes

            state["hook"] = _ntff_profile_via_ctypes("/opt/axon/libaxon_pjrt.so")
        except Exception:
            pass
        mod.get_axon_ntff_profile_hook = lambda: state["hook"]
        mod.set_axon_ntff_profile_hook = lambda h: state.__setitem__("hook", h)
        sys.modules["antenv.axon_hooks"] = mod
    from concourse import bass_utils as bu

    if not getattr(bu.upload_artifacts, "_actnet_safe", False):
        _orig = bu.upload_artifacts

        def _safe(tmpdir):
            try:
                return _orig(tmpdir)
            except Exception:
                return "local:" + tmpdir

        _safe._actnet_safe = True
        bu.upload_artifacts = _safe


def kernel(x, W1, b1, Wh, bh, Wout, bout):
    global LAST_RESULT
    from concourse.bass_utils import run_bass_kernel_spmd

    x = np.asarray(x, np.float32)
    B = x.shape[0]
    assert B % N_CORES == 0
    shard = B // N_CORES
    # pack x into the SBUF image layout: [24, shard/4] per core, where row
    # 6*i+f holds feature f of the blocks on partition-strip i
    x5 = x.reshape(N_CORES, shard // OCT_ROWS, 2, 4, NBLK, 6)  # c,q,h,i,n,f
    xq = np.ascontiguousarray(x5.transpose(0, 3, 5, 1, 2, 4)).reshape(
        N_CORES, 24, shard // 4
    )

    if ("nc", shard) not in _cache:
        _cache[("nc", shard)] = _build(shard)
    nc = _cache[("nc", shard)]

    wpack = _pack_weights(
        np.asarray(W1, np.float32),
        np.asarray(b1, np.float32),
        np.asarray(Wh, np.float32),
        np.asarray(bh, np.float32),
        np.asarray(Wout, np.float32),
        np.asarray(bout, np.float32),
    )
    in_maps = [{"xq": xq[c], **wpack} for c in range(N_CORES)]
    trace = bool(os.environ.get("ACTNET_TRACE"))
    if trace:
        _install_ntff_hook()
    res = run_bass_kernel_spmd(
        nc, in_maps, list(range(N_CORES)), trace=trace
    )
    LAST_RESULT = res
    out = np.concatenate([res.results[c]["out"] for c in range(N_CORES)], axis=0)
    return out.astype(np.float32)


if __name__ == "__main__":
    # smoke test with random data
    rng = np.random.default_rng(0)
    B = B_FULL
    inputs = dict(
        x=rng.standard_normal((B, 6), dtype=np.float32),
        W1=(rng.standard_normal((6, 32)) / np.sqrt(6)).astype(np.float32),
        b1=(rng.standard_normal(32) * 0.01).astype(np.float32),
        Wh=(rng.standard_normal((19, 32, 32)) / np.sqrt(32)).astype(np.float32),
        bh=(rng.standard_normal((19, 32)) * 0.01).astype(np.float32),
        Wout=(rng.standard_normal((32, 1)) / np.sqrt(32)).astype(np.float32),
        bout=(rng.standard_normal(1) * 0.01).astype(np.float32),
    )
    y = kernel(**inputs)
    print("kernel out", y.shape, y.dtype, y[:4, 0])
